# revision 33
# baseline (speedup 1.0000x reference)
"""Trainium2 Bass kernel for BoundingBoxRegression (topk_masking).

kernel(**inputs) takes FULL unsharded numpy inputs, returns the FULL output
tuple (maps, bbox) matching the reference:
  maps = concat([score_m, w_m, h_m], axis=1) -> (16, 192, 128, 128) f32
  bbox -> (16*64, 6) f32

Sharding: pure data parallel, batch 16 -> 2 per core on 8 cores.

Device computes the three masked maps (99.99% of output bytes / FLOPs):
  - depthwise 11-tap convs as folded-Toeplitz fp32 matmuls on PE
    (per-channel scales w_bbx * {H,W} * *_sh folded into the Toeplitz)
  - top-k score mask applied as x > t_c with per-(b,c) thresholds
    computed on host (exact f32 reproduction of
    sigmoid(x) > max(sigmoid(x)) - 0.01 via threshold inversion)
  - channel-max masking (v * (v == max_c v)) of all three maps on device

Host computes the Toeplitz fold, the mask thresholds, and the tiny bbox
reduction epilogue (~4 masked pixels per (b,c) map).
"""

import numpy as np

B, C, H, W = 16, 64, 128, 128
NCORES = 8
BPC = B // NCORES   # batches per core
KT = 11             # conv taps
CW = C * W          # free width of the big SBUF slabs (8192)

GC = 4              # channels per PE/PSUM group
NG = C // GC        # 16 groups
MC = 8              # channels per masking/score chunk
NM = C // MC        # 8 chunks


# ---------------------------------------------------------------------------
# Host-side numerics
# ---------------------------------------------------------------------------

def _host_prep(x, w_bbx, w_width, w_width_sh, w_height, w_height_sh):
    """Returns (TH_sb, TW_sb, thr, bbox).

    TH_sb/TW_sb: [128, C*128] f32 SBUF-layout folded Toeplitz matrices.
    thr:         [B, C] f32 mask thresholds (device mask = x > thr).
    bbox:        [B*C, 6] f32 final bbox output.
    """
    xf = x.astype(np.float32, copy=False)
    bbx = w_bbx.reshape(C).astype(np.float64)
    tw = w_width.reshape(C, KT).astype(np.float64)
    tws = w_width_sh.reshape(C).astype(np.float64)
    th = w_height.reshape(C, KT).astype(np.float64)
    ths = w_height_sh.reshape(C).astype(np.float64)

    # out[:, j] = sum_k x[:, j+k-5] * tap[k] * scale
    # -> T[i, j] = tap[i-j+5] * scale (i = source index, j = dest index)
    scw = bbx * W * tws
    sch = bbx * H * ths
    TW = np.zeros((C, W, W), dtype=np.float32)
    TH = np.zeros((C, H, H), dtype=np.float32)
    for k in range(KT):
        d = k - 5
        j = np.arange(max(0, -d), min(W, W - d))
        for c in range(C):
            TW[c, j + d, j] = np.float32(tw[c, k] * scw[c])
            TH[c, j + d, j] = np.float32(th[c, k] * sch[c])
    TW_sb = np.ascontiguousarray(TW.transpose(1, 0, 2).reshape(W, C * W))
    TH_sb = np.ascontiguousarray(TH.transpose(1, 0, 2).reshape(H, C * H))

    # Mask thresholds: reproduce sigmoid(x) > max(sigmoid(x)) - 0.01.
    # Use jax on its default backend so the mask matches the reference as
    # executed in this environment (sigmoid rounding differs per backend).
    mask = None
    sxmax = None
    try:
        import jax
        import jax.numpy as jnp
        xj = jnp.asarray(xf)
        sxj = jax.nn.sigmoid(xj)
        Mj = jnp.max(sxj, axis=(-2, -1), keepdims=True) - 0.01
        mask_j = np.asarray(sxj > Mj)
        sxmax_j = np.asarray(jnp.max(sxj, axis=(-2, -1)))
        # threshold inversion is only exact if the mask is upward-closed
        # in x per (b, c); verify.
        xin = np.where(mask_j, xf, np.inf).min(axis=(2, 3))
        xout = np.where(mask_j, -np.inf, xf).max(axis=(2, 3))
        if np.all(xin > xout):
            mask = mask_j
            sxmax = sxmax_j
    except Exception:
        pass
    if mask is None:
        sx = (1.0 / (1.0 + np.exp(-xf.astype(np.float64)))).astype(np.float32)
        sxmax = sx.max(axis=(2, 3))
        Mthr = (sxmax - np.float32(0.01)).astype(np.float32)
        mask = sx > Mthr[:, :, None, None]
    thr = np.where(mask, -np.inf, xf).max(axis=(2, 3)).astype(np.float32)

    # Score-map channel-max collisions: pixels where >=2 channels are
    # masked. Device writes the raw score map; the loser channels (score
    # < channel max) must be zeroed. Host knows them exactly because the
    # device score values are bitwise equal to x at masked pixels.
    coll = []
    nnzc = mask.sum(axis=1)
    for b, i, j in np.argwhere(nnzc >= 2):
        cs = np.where(mask[b, :, i, j])[0]
        xv = xf[b, cs, i, j]
        for c in cs[xv < xv.max()]:
            coll.append((b, c, i, j))

    # ---- bbox epilogue (sparse: ~4 masked pixels per (b,c) map) ----
    x64 = xf.astype(np.float64)
    score = np.where(mask, x64, 0.0)
    S = score.sum(axis=(2, 3))
    col = np.arange(W, dtype=np.float64)
    row = np.arange(H, dtype=np.float64)
    Scol = (score * col[None, None, None, :]).sum(axis=(2, 3))
    Srow = (score * row[None, None, :, None]).sum(axis=(2, 3))

    xp_w = np.pad(x64, ((0, 0), (0, 0), (0, 0), (5, 5)))
    xp_h = np.pad(x64, ((0, 0), (0, 0), (5, 5), (0, 0)))
    Sw = np.zeros((B, C))
    Sh = np.zeros((B, C))
    for b, c, i, j in zip(*np.nonzero(mask)):
        wv = np.dot(xp_w[b, c, i, j:j + KT], tw[c]) * scw[c]
        hv = np.dot(xp_h[b, c, i:i + KT, j], th[c]) * sch[c]
        Sw[b, c] += wv * x64[b, c, i, j]
        Sh[b, c] += hv * x64[b, c, i, j]

    ws = Sw / S
    hs = Sh / S
    x1 = Scol / S - ws / 2
    y1 = Srow / S - hs / 2
    ivec = np.broadcast_to(np.arange(B, dtype=np.float64)[:, None], (B, C))
    bbox = np.stack(
        [ivec, x1, y1, x1 + ws, y1 + hs,
         np.asarray(sxmax).astype(np.float64)],
        axis=-1).reshape(-1, 6).astype(np.float32)
    return TH_sb, TW_sb, thr, bbox, coll


# ---------------------------------------------------------------------------
# Device program (SPMD, one NeuronCore's share: 2 batches)
# ---------------------------------------------------------------------------

def _build_program():
    import concourse.tile as tile
    from concourse import bacc, mybir

    fp32 = mybir.dt.float32
    AX = mybir.AxisListType
    OP = mybir.AluOpType

    nc = bacc.Bacc("TRN2", target_bir_lowering=False, debug=False)

    x_d = nc.dram_tensor("x", [BPC, C, H, W], fp32, kind="ExternalInput")
    th_d = nc.dram_tensor("th", [H, CW], fp32, kind="ExternalInput")
    tw_d = nc.dram_tensor("tw", [W, CW], fp32, kind="ExternalInput")
    thr_d = nc.dram_tensor("thr", [128, BPC * C], fp32, kind="ExternalInput")
    id_d = nc.dram_tensor("ident", [128, 128], fp32, kind="ExternalInput")
    maps_d = nc.dram_tensor("maps", [BPC, 3 * C, H, W], fp32,
                            kind="ExternalOutput")

    with tile.TileContext(nc) as tc:
        with (
            tc.tile_pool(name="const", bufs=1) as cpool,
            tc.tile_pool(name="xa", bufs=6) as xapool,
            tc.tile_pool(name="wa", bufs=8) as wapool,
            tc.tile_pool(name="ha", bufs=8) as hapool,
            tc.tile_pool(name="sc", bufs=3) as scpool,
            tc.tile_pool(name="xta", bufs=2) as xtapool,
            tc.tile_pool(name="wta", bufs=2) as wtapool,
            tc.tile_pool(name="scr", bufs=3) as scrpool,
            tc.tile_pool(name="cmax", bufs=1) as cmpool,
            tc.tile_pool(name="ppx", bufs=2, space="PSUM") as ppx,
            tc.tile_pool(name="pph", bufs=2, space="PSUM") as pph,
            tc.tile_pool(name="ppwt", bufs=2, space="PSUM") as ppwt,
            tc.tile_pool(name="ppw", bufs=2, space="PSUM") as ppw,
        ):
            TH = cpool.tile([H, CW], fp32, tag="TH")
            TW = cpool.tile([W, CW], fp32, tag="TW")
            ident = cpool.tile([128, 128], fp32, tag="ident")
            tbc = cpool.tile([128, BPC * C], fp32, tag="tbc")
            nc.sync.dma_start(ident[:, :], id_d.ap()[:, :])
            nc.sync.dma_start(tbc[:, :], thr_d.ap()[:, :])

            SL = 8             # slices per map buffer
            SCH = C // SL      # channels per slice (8)
            for b in range(BPC):
                # rolling slice-granular map buffers: batch b+1's evacs
                # into slice s unblock as soon as batch b's masking of
                # slice s is stored, instead of waiting for the whole map.
                wa_t = [wapool.tile([128, SCH * W], fp32, tag="wa",
                                    name=f"wa_{b}_{s}")
                        for s in range(SL)]
                ha_t = [hapool.tile([128, SCH * W], fp32, tag="ha",
                                    name=f"ha_{b}_{s}")
                        for s in range(SL)]
                CMW = cmpool.tile([128, 128], fp32, tag="cmw")
                CMH = cmpool.tile([128, 128], fp32, tag="cmh")

                # batch>=1: hoist all x loads ahead of the group loop so
                # on the ACT queue they issue before any evac that waits
                # on the previous batch's masking.
                xas = {}
                if b > 0:
                    for g in range(NG):
                        xa_i = xapool.tile([128, GC * 128], fp32, tag="xa",
                                           name=f"xa_{b}_{g}")
                        nc.scalar.dma_start(
                            xa_i[:, :].rearrange("p (c w) -> p c w", w=W),
                            x_d.ap()[b, g * GC:(g + 1) * GC, :, :]
                            .rearrange("c h w -> h c w"))
                        xas[g] = xa_i

                # conv + score pipeline over rotating 4-channel x slabs;
                # back-transpose of the width branch happens inside each
                # group so all three maps finish with the conv phase, and
                # the x slab retires as soon as its group is done.
                for g in range(NG):
                    c0 = g * GC
                    sl = g // 2
                    so = (g % 2) * GC * W  # column offset within slice
                    if b == 0:
                        # pace the Toeplitz loads with the groups that
                        # consume them (one big DMA would stall group 0)
                        nc.sync.dma_start(
                            TH[:, c0 * 128:(c0 + GC) * 128],
                            th_d.ap()[:, c0 * 128:(c0 + GC) * 128])
                        nc.sync.dma_start(
                            TW[:, c0 * 128:(c0 + GC) * 128],
                            tw_d.ap()[:, c0 * 128:(c0 + GC) * 128])
                    if b == 0:
                        XA = xapool.tile([128, GC * 128], fp32, tag="xa")
                        nc.sync.dma_start(
                            XA[:, :].rearrange("p (c w) -> p c w", w=W),
                            x_d.ap()[b, c0:c0 + GC, :, :]
                            .rearrange("c h w -> h c w"))
                    else:
                        XA = xas[g]
                    XA3 = XA[:, :].rearrange("p (c w) -> p c w", w=W)

                    px = ppx.tile([128, GC * 128], fp32, tag="px")
                    for j in range(GC):
                        nc.tensor.transpose(
                            px[:, j * 128:(j + 1) * 128],
                            XA3[:, j, :], ident[:, :])
                    XTA = xtapool.tile([128, GC * 128], fp32, tag="xta")
                    nc.scalar.copy(XTA[:, :], px[:, :])

                    ph = pph.tile([128, GC * 128], fp32, tag="ph")
                    for j in range(GC):
                        c = c0 + j
                        nc.tensor.matmul(
                            ph[:, j * 128:(j + 1) * 128],
                            TH[:, c * 128:(c + 1) * 128],
                            XA3[:, j, :])
                    nc.scalar.copy(ha_t[sl][:, so:so + GC * W], ph[:, :])

                    pwt = ppwt.tile([128, GC * 128], fp32, tag="pwt")
                    for j in range(GC):
                        c = c0 + j
                        nc.tensor.matmul(
                            pwt[:, j * 128:(j + 1) * 128],
                            TW[:, c * 128:(c + 1) * 128],
                            XTA[:, j * 128:(j + 1) * 128])
                    WTA = wtapool.tile([128, GC * 128], fp32, tag="wta")
                    nc.scalar.copy(WTA[:, :], pwt[:, :])

                    pw = ppw.tile([128, GC * 128], fp32, tag="pw")
                    for j in range(GC):
                        nc.tensor.transpose(
                            pw[:, j * 128:(j + 1) * 128],
                            WTA[:, j * 128:(j + 1) * 128], ident[:, :])
                    nc.scalar.copy(wa_t[sl][:, so:so + GC * W], pw[:, :])

                    # score for this slab: mask = x > t (DVE),
                    # score = mask * x (POOL). Channel-max masking of the
                    # score map happens on host (only a handful of
                    # collision pixels differ).
                    scr = scrpool.tile([128, GC * 128], fp32, tag="scr")
                    scr3 = scr[:, :].rearrange("p (c w) -> p c w", w=W)
                    sc = scpool.tile([128, GC * 128], fp32, tag="sc")
                    sc3 = sc[:, :].rearrange("p (c w) -> p c w", w=W)
                    tin = (tbc[:, b * C + c0: b * C + c0 + GC]
                           .unsqueeze(2).broadcast_to([128, GC, W]))
                    nc.vector.tensor_tensor(scr3, XA3, tin, op=OP.is_gt)
                    nc.gpsimd.tensor_tensor(sc3, scr3, XA3, op=OP.mult)
                    nc.sync.dma_start(
                        maps_d.ap()[b, c0:c0 + GC, :, :]
                        .rearrange("c h w -> h c w"), sc3)

                    # per-slice channel-max partials overlap the conv phase
                    if g % 2 == 1:
                        for tiles, CM in ((ha_t, CMH), (wa_t, CMW)):
                            src = (tiles[sl][:, :]
                                   .rearrange("p (c w) -> p w c", w=W))
                            if sl == 0:
                                nc.vector.tensor_reduce(
                                    CM[:, :], src, axis=AX.X, op=OP.max)
                            else:
                                tmp = cmpool.tile([128, 128], fp32,
                                                  tag="cmt")
                                nc.vector.tensor_reduce(
                                    tmp[:, :], src, axis=AX.X, op=OP.max)
                                nc.vector.tensor_tensor(
                                    CM[:, :], CM[:, :], tmp[:, :],
                                    op=OP.max)

                # apply (v >= cmax) masks in place, then store.
                # W/H slices interleaved so both buffers drain early;
                # compare on DVE; multiply mostly on POOL for balance.
                mi = 0
                for m in range(SL):
                    for tiles, cm, off in (
                        (wa_t, CMW, C),
                        (ha_t, CMH, 2 * C),
                    ):
                        buf3 = (tiles[m][:, :]
                                .rearrange("p (c w) -> p c w", w=W))
                        c0 = m * SCH
                        scr = scrpool.tile([128, SCH * W], fp32, tag="scr")
                        scr3 = scr[:, :].rearrange("p (c w) -> p c w", w=W)
                        cmb = (cm[:, :].unsqueeze(1)
                               .broadcast_to([128, SCH, W]))
                        nc.vector.tensor_tensor(
                            scr3, buf3, cmb, op=OP.is_ge)
                        mul_eng = nc.vector if mi % 3 == 2 else nc.gpsimd
                        mul_eng.tensor_tensor(buf3, buf3, scr3, op=OP.mult)
                        mi += 1
                        nc.sync.dma_start(
                            maps_d.ap()[b, off + c0: off + c0 + SCH, :, :]
                            .rearrange("c h w -> h c w"), buf3)
    nc.compile()
    return nc


_PROGRAM = None


def kernel(x, w_bbx, w_width, w_width_sh, w_height, w_height_sh):
    global _PROGRAM
    x = np.asarray(x, dtype=np.float32)
    TH_sb, TW_sb, thr, bbox, coll = _host_prep(
        x, np.asarray(w_bbx), np.asarray(w_width),
        np.asarray(w_width_sh), np.asarray(w_height),
        np.asarray(w_height_sh))

    from concourse.bass_utils import run_bass_kernel_spmd

    if _PROGRAM is None:
        _PROGRAM = _build_program()
    nc = _PROGRAM

    ident = np.eye(128, dtype=np.float32)
    in_maps = []
    for i in range(NCORES):
        thr_i = np.ascontiguousarray(
            np.broadcast_to(thr[i * BPC:(i + 1) * BPC].reshape(1, -1),
                            (128, BPC * C)))
        in_maps.append({
            "x": np.ascontiguousarray(x[i * BPC:(i + 1) * BPC]),
            "th": TH_sb,
            "tw": TW_sb,
            "thr": thr_i,
            "ident": ident,
        })
    res = run_bass_kernel_spmd(nc, in_maps, list(range(NCORES)))
    maps = np.concatenate(
        [res.results[i]["maps"] for i in range(NCORES)], axis=0)
    # zero the score-map channel-max collision losers (host-exact)
    for b, c, i, j in coll:
        maps[b, c, i, j] = 0.0
    return maps, bbox


# revision 34
# speedup vs baseline: 1.0461x; 1.0461x over previous
"""Trainium2 Bass kernel for BoundingBoxRegression (topk_masking).

kernel(**inputs) takes FULL unsharded numpy inputs, returns the FULL output
tuple (maps, bbox) matching the reference:
  maps = concat([score_m, w_m, h_m], axis=1) -> (16, 192, 128, 128) f32
  bbox -> (16*64, 6) f32

Sharding: pure data parallel, batch 16 -> 2 per core on 8 cores.

Device computes the three masked maps (99.99% of output bytes / FLOPs):
  - depthwise 11-tap convs as folded-Toeplitz fp32 matmuls on PE
    (per-channel scales w_bbx * {H,W} * *_sh folded into the Toeplitz)
  - top-k score mask applied as x > t_c with per-(b,c) thresholds
    computed on host (exact f32 reproduction of
    sigmoid(x) > max(sigmoid(x)) - 0.01 via threshold inversion)
  - channel-max masking (v * (v == max_c v)) of all three maps on device

Host computes the Toeplitz fold, the mask thresholds, and the tiny bbox
reduction epilogue (~4 masked pixels per (b,c) map).
"""

import numpy as np

B, C, H, W = 16, 64, 128, 128
NCORES = 8
BPC = B // NCORES   # batches per core
KT = 11             # conv taps
CW = C * W          # free width of the big SBUF slabs (8192)

GC = 4              # channels per PE/PSUM group
NG = C // GC        # 16 groups
MC = 8              # channels per masking/score chunk
NM = C // MC        # 8 chunks


# ---------------------------------------------------------------------------
# Host-side numerics
# ---------------------------------------------------------------------------

def _host_prep(x, w_bbx, w_width, w_width_sh, w_height, w_height_sh):
    """Returns (TH_sb, TW_sb, thr, bbox).

    TH_sb/TW_sb: [128, C*128] f32 SBUF-layout folded Toeplitz matrices.
    thr:         [B, C] f32 mask thresholds (device mask = x > thr).
    bbox:        [B*C, 6] f32 final bbox output.
    """
    xf = x.astype(np.float32, copy=False)
    bbx = w_bbx.reshape(C).astype(np.float64)
    tw = w_width.reshape(C, KT).astype(np.float64)
    tws = w_width_sh.reshape(C).astype(np.float64)
    th = w_height.reshape(C, KT).astype(np.float64)
    ths = w_height_sh.reshape(C).astype(np.float64)

    # out[:, j] = sum_k x[:, j+k-5] * tap[k] * scale
    # -> T[i, j] = tap[i-j+5] * scale (i = source index, j = dest index)
    scw = bbx * W * tws
    sch = bbx * H * ths
    TW = np.zeros((C, W, W), dtype=np.float32)
    TH = np.zeros((C, H, H), dtype=np.float32)
    for k in range(KT):
        d = k - 5
        j = np.arange(max(0, -d), min(W, W - d))
        for c in range(C):
            TW[c, j + d, j] = np.float32(tw[c, k] * scw[c])
            TH[c, j + d, j] = np.float32(th[c, k] * sch[c])
    TW_sb = np.ascontiguousarray(TW.transpose(1, 0, 2).reshape(W, C * W))
    TH_sb = np.ascontiguousarray(TH.transpose(1, 0, 2).reshape(H, C * H))

    # Mask thresholds: reproduce sigmoid(x) > max(sigmoid(x)) - 0.01.
    # Use jax on its default backend so the mask matches the reference as
    # executed in this environment (sigmoid rounding differs per backend).
    mask = None
    sxmax = None
    try:
        import jax
        import jax.numpy as jnp
        xj = jnp.asarray(xf)
        sxj = jax.nn.sigmoid(xj)
        Mj = jnp.max(sxj, axis=(-2, -1), keepdims=True) - 0.01
        mask_j = np.asarray(sxj > Mj)
        sxmax_j = np.asarray(jnp.max(sxj, axis=(-2, -1)))
        # threshold inversion is only exact if the mask is upward-closed
        # in x per (b, c); verify.
        xin = np.where(mask_j, xf, np.inf).min(axis=(2, 3))
        xout = np.where(mask_j, -np.inf, xf).max(axis=(2, 3))
        if np.all(xin > xout):
            mask = mask_j
            sxmax = sxmax_j
    except Exception:
        pass
    if mask is None:
        sx = (1.0 / (1.0 + np.exp(-xf.astype(np.float64)))).astype(np.float32)
        sxmax = sx.max(axis=(2, 3))
        Mthr = (sxmax - np.float32(0.01)).astype(np.float32)
        mask = sx > Mthr[:, :, None, None]
    thr = np.where(mask, -np.inf, xf).max(axis=(2, 3)).astype(np.float32)

    # Score-map channel-max collisions: pixels where >=2 channels are
    # masked. Device writes the raw score map; the loser channels (score
    # < channel max) must be zeroed. Host knows them exactly because the
    # device score values are bitwise equal to x at masked pixels.
    coll = []
    nnzc = mask.sum(axis=1)
    for b, i, j in np.argwhere(nnzc >= 2):
        cs = np.where(mask[b, :, i, j])[0]
        xv = xf[b, cs, i, j]
        for c in cs[xv < xv.max()]:
            coll.append((b, c, i, j))

    # ---- bbox epilogue (sparse: ~4 masked pixels per (b,c) map) ----
    x64 = xf.astype(np.float64)
    score = np.where(mask, x64, 0.0)
    S = score.sum(axis=(2, 3))
    col = np.arange(W, dtype=np.float64)
    row = np.arange(H, dtype=np.float64)
    Scol = (score * col[None, None, None, :]).sum(axis=(2, 3))
    Srow = (score * row[None, None, :, None]).sum(axis=(2, 3))

    xp_w = np.pad(x64, ((0, 0), (0, 0), (0, 0), (5, 5)))
    xp_h = np.pad(x64, ((0, 0), (0, 0), (5, 5), (0, 0)))
    Sw = np.zeros((B, C))
    Sh = np.zeros((B, C))
    for b, c, i, j in zip(*np.nonzero(mask)):
        wv = np.dot(xp_w[b, c, i, j:j + KT], tw[c]) * scw[c]
        hv = np.dot(xp_h[b, c, i:i + KT, j], th[c]) * sch[c]
        Sw[b, c] += wv * x64[b, c, i, j]
        Sh[b, c] += hv * x64[b, c, i, j]

    ws = Sw / S
    hs = Sh / S
    x1 = Scol / S - ws / 2
    y1 = Srow / S - hs / 2
    ivec = np.broadcast_to(np.arange(B, dtype=np.float64)[:, None], (B, C))
    bbox = np.stack(
        [ivec, x1, y1, x1 + ws, y1 + hs,
         np.asarray(sxmax).astype(np.float64)],
        axis=-1).reshape(-1, 6).astype(np.float32)
    return TH_sb, TW_sb, thr, bbox, coll


# ---------------------------------------------------------------------------
# Device program (SPMD, one NeuronCore's share: 2 batches)
# ---------------------------------------------------------------------------

def _build_program():
    import concourse.tile as tile
    from concourse import bacc, mybir

    fp32 = mybir.dt.float32
    AX = mybir.AxisListType
    OP = mybir.AluOpType

    nc = bacc.Bacc("TRN2", target_bir_lowering=False, debug=False)

    x_d = nc.dram_tensor("x", [BPC, C, H, W], fp32, kind="ExternalInput")
    th_d = nc.dram_tensor("th", [H, CW], fp32, kind="ExternalInput")
    tw_d = nc.dram_tensor("tw", [W, CW], fp32, kind="ExternalInput")
    thr_d = nc.dram_tensor("thr", [128, BPC * C], fp32, kind="ExternalInput")
    id_d = nc.dram_tensor("ident", [128, 128], fp32, kind="ExternalInput")
    maps_d = nc.dram_tensor("maps", [BPC, 3 * C, H, W], fp32,
                            kind="ExternalOutput")

    with tile.TileContext(nc) as tc:
        with (
            tc.tile_pool(name="const", bufs=1) as cpool,
            tc.tile_pool(name="xa", bufs=8) as xapool,
            tc.tile_pool(name="wa", bufs=8) as wapool,
            tc.tile_pool(name="ha", bufs=8) as hapool,
            tc.tile_pool(name="sc", bufs=3) as scpool,
            tc.tile_pool(name="xta", bufs=2) as xtapool,
            tc.tile_pool(name="wta", bufs=2) as wtapool,
            tc.tile_pool(name="scr", bufs=3) as scrpool,
            tc.tile_pool(name="cmax", bufs=1) as cmpool,
            tc.tile_pool(name="ppx", bufs=1, space="PSUM") as ppx,
            tc.tile_pool(name="pph", bufs=3, space="PSUM") as pph,
            tc.tile_pool(name="ppwt", bufs=1, space="PSUM") as ppwt,
            tc.tile_pool(name="ppw", bufs=3, space="PSUM") as ppw,
        ):
            TH = cpool.tile([H, CW], fp32, tag="TH")
            TW = cpool.tile([W, CW], fp32, tag="TW")
            ident = cpool.tile([128, 128], fp32, tag="ident")
            tbc = cpool.tile([128, BPC * C], fp32, tag="tbc")
            nc.sync.dma_start(ident[:, :], id_d.ap()[:, :])
            nc.sync.dma_start(tbc[:, :], thr_d.ap()[:, :])

            SL = 8             # slices per map buffer
            SCH = C // SL      # channels per slice (8)
            for b in range(BPC):
                # rolling slice-granular map buffers: batch b+1's evacs
                # into slice s unblock as soon as batch b's masking of
                # slice s is stored, instead of waiting for the whole map.
                wa_t = [wapool.tile([128, SCH * W], fp32, tag="wa",
                                    name=f"wa_{b}_{s}")
                        for s in range(SL)]
                ha_t = [hapool.tile([128, SCH * W], fp32, tag="ha",
                                    name=f"ha_{b}_{s}")
                        for s in range(SL)]
                CMW = cmpool.tile([128, 128], fp32, tag="cmw")
                CMH = cmpool.tile([128, 128], fp32, tag="cmh")

                # batch>=1: hoist all x loads ahead of the group loop so
                # on the ACT queue they issue before any evac that waits
                # on the previous batch's masking.
                xas = {}
                if b > 0:
                    for g in range(NG):
                        xa_i = xapool.tile([128, GC * 128], fp32, tag="xa",
                                           name=f"xa_{b}_{g}")
                        nc.scalar.dma_start(
                            xa_i[:, :].rearrange("p (c w) -> p c w", w=W),
                            x_d.ap()[b, g * GC:(g + 1) * GC, :, :]
                            .rearrange("c h w -> h c w"))
                        xas[g] = xa_i

                # conv + score pipeline over rotating 4-channel x slabs;
                # back-transpose of the width branch happens inside each
                # group so all three maps finish with the conv phase, and
                # the x slab retires as soon as its group is done.
                for g in range(NG):
                    c0 = g * GC
                    sl = g // 2
                    so = (g % 2) * GC * W  # column offset within slice
                    if b == 0:
                        # pace the Toeplitz loads with the groups that
                        # consume them (one big DMA would stall group 0)
                        nc.sync.dma_start(
                            TH[:, c0 * 128:(c0 + GC) * 128],
                            th_d.ap()[:, c0 * 128:(c0 + GC) * 128])
                        nc.sync.dma_start(
                            TW[:, c0 * 128:(c0 + GC) * 128],
                            tw_d.ap()[:, c0 * 128:(c0 + GC) * 128])
                    if b == 0:
                        XA = xapool.tile([128, GC * 128], fp32, tag="xa")
                        nc.sync.dma_start(
                            XA[:, :].rearrange("p (c w) -> p c w", w=W),
                            x_d.ap()[b, c0:c0 + GC, :, :]
                            .rearrange("c h w -> h c w"))
                    else:
                        XA = xas[g]
                    XA3 = XA[:, :].rearrange("p (c w) -> p c w", w=W)

                    px = ppx.tile([128, GC * 128], fp32, tag="px")
                    for j in range(GC):
                        nc.tensor.transpose(
                            px[:, j * 128:(j + 1) * 128],
                            XA3[:, j, :], ident[:, :])
                    XTA = xtapool.tile([128, GC * 128], fp32, tag="xta")
                    nc.scalar.copy(XTA[:, :], px[:, :])

                    ph = pph.tile([128, GC * 128], fp32, tag="ph")
                    for j in range(GC):
                        c = c0 + j
                        nc.tensor.matmul(
                            ph[:, j * 128:(j + 1) * 128],
                            TH[:, c * 128:(c + 1) * 128],
                            XA3[:, j, :])
                    nc.scalar.copy(ha_t[sl][:, so:so + GC * W], ph[:, :])

                    pwt = ppwt.tile([128, GC * 128], fp32, tag="pwt")
                    for j in range(GC):
                        c = c0 + j
                        nc.tensor.matmul(
                            pwt[:, j * 128:(j + 1) * 128],
                            TW[:, c * 128:(c + 1) * 128],
                            XTA[:, j * 128:(j + 1) * 128])
                    WTA = wtapool.tile([128, GC * 128], fp32, tag="wta")
                    nc.scalar.copy(WTA[:, :], pwt[:, :])

                    pw = ppw.tile([128, GC * 128], fp32, tag="pw")
                    for j in range(GC):
                        nc.tensor.transpose(
                            pw[:, j * 128:(j + 1) * 128],
                            WTA[:, j * 128:(j + 1) * 128], ident[:, :])
                    nc.scalar.copy(wa_t[sl][:, so:so + GC * W], pw[:, :])

                    # score for this slab: mask = x > t (DVE),
                    # score = mask * x (POOL). Channel-max masking of the
                    # score map happens on host (only a handful of
                    # collision pixels differ).
                    scr = scrpool.tile([128, GC * 128], fp32, tag="scr")
                    scr3 = scr[:, :].rearrange("p (c w) -> p c w", w=W)
                    sc = scpool.tile([128, GC * 128], fp32, tag="sc")
                    sc3 = sc[:, :].rearrange("p (c w) -> p c w", w=W)
                    tin = (tbc[:, b * C + c0: b * C + c0 + GC]
                           .unsqueeze(2).broadcast_to([128, GC, W]))
                    nc.vector.tensor_tensor(scr3, XA3, tin, op=OP.is_gt)
                    nc.gpsimd.tensor_tensor(sc3, scr3, XA3, op=OP.mult)
                    nc.sync.dma_start(
                        maps_d.ap()[b, c0:c0 + GC, :, :]
                        .rearrange("c h w -> h c w"), sc3)

                    # per-slice channel-max partials overlap the conv phase
                    if g % 2 == 1:
                        for tiles, CM in ((ha_t, CMH), (wa_t, CMW)):
                            src = (tiles[sl][:, :]
                                   .rearrange("p (c w) -> p w c", w=W))
                            if sl == 0:
                                nc.vector.tensor_reduce(
                                    CM[:, :], src, axis=AX.X, op=OP.max)
                            else:
                                tmp = cmpool.tile([128, 128], fp32,
                                                  tag="cmt")
                                nc.vector.tensor_reduce(
                                    tmp[:, :], src, axis=AX.X, op=OP.max)
                                nc.vector.tensor_tensor(
                                    CM[:, :], CM[:, :], tmp[:, :],
                                    op=OP.max)

                # apply (v >= cmax) masks in place, then store.
                # W/H slices interleaved so both buffers drain early;
                # compare on DVE; multiply mostly on POOL for balance.
                mi = 0
                for m in range(SL):
                    for tiles, cm, off in (
                        (wa_t, CMW, C),
                        (ha_t, CMH, 2 * C),
                    ):
                        buf3 = (tiles[m][:, :]
                                .rearrange("p (c w) -> p c w", w=W))
                        c0 = m * SCH
                        scr = scrpool.tile([128, SCH * W], fp32, tag="scr")
                        scr3 = scr[:, :].rearrange("p (c w) -> p c w", w=W)
                        cmb = (cm[:, :].unsqueeze(1)
                               .broadcast_to([128, SCH, W]))
                        nc.vector.tensor_tensor(
                            scr3, buf3, cmb, op=OP.is_ge)
                        mul_eng = nc.vector if mi % 3 == 2 else nc.gpsimd
                        mul_eng.tensor_tensor(buf3, buf3, scr3, op=OP.mult)
                        mi += 1
                        nc.sync.dma_start(
                            maps_d.ap()[b, off + c0: off + c0 + SCH, :, :]
                            .rearrange("c h w -> h c w"), buf3)
    nc.compile()
    return nc


_PROGRAM = None


def kernel(x, w_bbx, w_width, w_width_sh, w_height, w_height_sh):
    global _PROGRAM
    x = np.asarray(x, dtype=np.float32)
    TH_sb, TW_sb, thr, bbox, coll = _host_prep(
        x, np.asarray(w_bbx), np.asarray(w_width),
        np.asarray(w_width_sh), np.asarray(w_height),
        np.asarray(w_height_sh))

    from concourse.bass_utils import run_bass_kernel_spmd

    if _PROGRAM is None:
        _PROGRAM = _build_program()
    nc = _PROGRAM

    ident = np.eye(128, dtype=np.float32)
    in_maps = []
    for i in range(NCORES):
        thr_i = np.ascontiguousarray(
            np.broadcast_to(thr[i * BPC:(i + 1) * BPC].reshape(1, -1),
                            (128, BPC * C)))
        in_maps.append({
            "x": np.ascontiguousarray(x[i * BPC:(i + 1) * BPC]),
            "th": TH_sb,
            "tw": TW_sb,
            "thr": thr_i,
            "ident": ident,
        })
    res = run_bass_kernel_spmd(nc, in_maps, list(range(NCORES)))
    maps = np.concatenate(
        [res.results[i]["maps"] for i in range(NCORES)], axis=0)
    # zero the score-map channel-max collision losers (host-exact)
    for b, c, i, j in coll:
        maps[b, c, i, j] = 0.0
    return maps, bbox
